# revision 20
# baseline (speedup 1.0000x reference)
"""Confusion-matrix kernel for Trainium2 (8 NeuronCores, data-parallel over batch).

Per batch b (one per core):
    pred[n]  = argmax_c input[b, c, n]            (n = pixel, N = H*W)
    cm[i, j] = sum_n target[b, i, n] * (pred[n] == j)
    rs[i]    = sum_n target[b, i, n]
Host: cm_b = cm / (rs + 1e-8); out = mean_b cm_b.

CLASS-MAJOR layout: per group (128 partitions x 6 pixel slots = 768 pixels),
x is stored as 21 class-blocks of 6 slot values -> 126 fp16 cols/group (no
pad column). The per-pixel max runs as an (8,5-inplace,4,2,1) block tree in
2x-mode tensor_tensor ops (60 cyc/group):
    T1: t8       = max(x[b0:b8],  x[b8:b16])
    T2: t8[0:5b] = max(t8[0:5b],  x[b16:b21])   (in-place accumulate)
    T3: t4       = max(t8[b0:b4], t8[b4:b8])
    T4: t2       = max(t4[b0:b2], t4[b2:b4])
    T5: m1       = max(t2[b0],    t2[b1])    -> per-slot max, 6 cols
    h[g, j, s] = is_ge(x[g, j, s], m1[g, s])  (63 cyc/group; bcast keeps 2x)
h is 132 cols/group: 126 one-hot + 6 ones cols (rs via matmul). The is_ge of
tile t-1 is woven between T2(t) and T3(t) so the dependent-chain DRAIN of
T2 overlaps an independent op.

One matmul per group: lhsT = y block [128, 128] fp8e4 (126 used),
rhs = h block [128, 132] fp16, accumulated into one [128, 132] f32 PSUM
tile; DVE does the final PSUM->SBUF copy. Host extracts
cm[i,j] = sum_s out[i*6+s, j*6+s], rs[i] = sum_s out[i*6+s, 126+s].

Buffering: x 4-deep, y 3-deep, h 3-deep; first y-load deferred until
isge(0) so the x stream gets full DMA bandwidth during the ramp. x/y DMA
completions are tracked by single cumulative semaphores (FIFO per queue);
5 semaphores total keeps the postamble sem-clear chain short.

Engines: SP x-loads + out-store | ACT y-loads | GPSIMD h ones-init |
DVE tree + is_ge + final copy | PE matmuls.
"""

from contextlib import ExitStack

import ml_dtypes
import numpy as np

import concourse.bass as bass
import concourse.mybir as mybir
from concourse.bass_utils import run_bass_kernel_spmd

B, C, H, W = 8, 21, 512, 512
N = H * W              # 262144 pixels per batch
P = 128                # SBUF partitions
S = 6                  # pixel slots per PE row
CW = C * S             # x cols per group (126), class-major
YW = 128               # y group width: 126 + 2 pad cols
HWC = (C + 1) * S      # h cols per group (132): 126 one-hot + 6 ones
NG = 342               # total groups per core (342*768 = 262656 >= N)
NPAD = NG * P * S      # padded pixel count
G_TILES = [10, 22, 44, 57, 57, 57, 57, 26, 12]
NT = len(G_TILES)
G_OFF = [sum(G_TILES[:i]) for i in range(NT)]
GMAX = max(G_TILES)
NEG = -65504.0
N_CORES = 8

_CACHED_NC = None


def build_nc():
    nc = bass.Bass()
    x = nc.declare_dram_parameter("x", [P, NG * CW], mybir.dt.float16, isOutput=False)
    y = nc.declare_dram_parameter("y", [P, NG * YW], mybir.dt.float8e4, isOutput=False)
    out = nc.declare_dram_parameter("out", [P, HWC], mybir.dt.float32, isOutput=True)

    mx = mybir.AluOpType.max

    with ExitStack() as ctx:
        xs = [
            ctx.enter_context(
                nc.sbuf_tensor(f"xsb{i}", [P, GMAX * CW], mybir.dt.float16)
            )
            for i in range(4)
        ]
        ys = [
            ctx.enter_context(
                nc.sbuf_tensor(f"ysb{i}", [P, GMAX * YW], mybir.dt.float8e4)
            )
            for i in range(3)
        ]
        hs = [
            ctx.enter_context(
                nc.sbuf_tensor(f"hsb{i}", [P, GMAX * HWC], mybir.dt.float16)
            )
            for i in range(3)
        ]
        t8 = ctx.enter_context(nc.sbuf_tensor("t8b", [P, GMAX * 48], mybir.dt.float16))
        t4 = ctx.enter_context(nc.sbuf_tensor("t4b", [P, GMAX * 24], mybir.dt.float16))
        t2 = ctx.enter_context(nc.sbuf_tensor("t2b", [P, GMAX * 12], mybir.dt.float16))
        m1s = [
            ctx.enter_context(nc.sbuf_tensor(f"m1b{i}", [P, GMAX * 6], mybir.dt.float16))
            for i in range(2)
        ]
        osb = ctx.enter_context(nc.sbuf_tensor("osb", [P, HWC], mybir.dt.float32))
        cm_psum = ctx.enter_context(nc.psum_tensor("cmps", [P, HWC], mybir.dt.float32))

        block = ctx.enter_context(nc.Block())
        sx = ctx.enter_context(nc.semaphore("sx"))      # x loads done (FIFO), 16/tile
        sy = ctx.enter_context(nc.semaphore("sy"))      # y loads done (FIFO), 16/tile
        shd = ctx.enter_context(nc.semaphore("shd"))    # DVE isge(t) done, = t+1
        sp = ctx.enter_context(nc.semaphore("sp"))      # PE tile matmuls done, = t+1
        aux = ctx.enter_context(nc.semaphore("aux"))    # 1=ones-init, 2=+copy, 18=+out-DMA

        def mview(buf, w, t):
            return (
                buf[:]
                .rearrange("p (g c) -> p g c", c=w)[:, 0 : G_TILES[t], :]
            )

        @block.sync
        def _(sync):
            for t in range(NT):
                if t >= 4:
                    sync.wait_ge(shd, t - 3)  # isge(t-4) freed x slot
                cols = G_TILES[t] * CW
                sync.dma_start(
                    out=xs[t % 4][:, 0:cols],
                    in_=x[:, G_OFF[t] * CW : G_OFF[t] * CW + cols],
                ).then_inc(sx, 16)
            sync.wait_ge(aux, 2)
            sync.dma_start(out=out[:], in_=osb[:]).then_inc(aux, 16)
            sync.wait_ge(aux, 18)

        @block.scalar
        def _(scalar):
            # defer the first y-loads until isge(1) is done: x feeds the DVE
            # ramp and gets the full DMA bandwidth until then (PE's per-tile
            # slack absorbs the later start).
            scalar.wait_ge(shd, 2)
            for t in range(NT):
                if t >= 3:
                    scalar.wait_ge(sp, t - 2)  # matmul(t-3) freed y slot
                cols = G_TILES[t] * YW
                scalar.dma_start(
                    out=ys[t % 3][:, 0:cols],
                    in_=y[:, G_OFF[t] * YW : G_OFF[t] * YW + cols],
                ).then_inc(sy, 16)


        @block.gpsimd
        def _(gpsimd):
            h3v = [h[:].rearrange("p (g w) -> p g w", w=HWC) for h in hs]
            nc.gpsimd.memset(h3v[0][:, :, CW:HWC], 1.0)
            nc.gpsimd.memset(h3v[1][:, :, CW:HWC], 1.0)
            nc.gpsimd.memset(h3v[2][:, :, CW:HWC], 1.0).then_inc(aux, 1)

        @block.vector
        def _(vector):
            def isge(t):
                # h(t) one-hot: compare x against per-slot max, broadcast
                # over the 21 class blocks (inner step 1 keeps 2x mode)
                if t >= 3:
                    vector.wait_ge(sp, t - 2)  # matmul(t-3) freed h slot
                G = G_TILES[t]
                x4 = (
                    xs[t % 4][:]
                    .rearrange("p (g j s) -> p g j s", j=C, s=S)[:, 0:G, :, :]
                )
                h4 = (
                    hs[t % 3][:]
                    .rearrange("p (g j s) -> p g j s", j=C + 1, s=S)
                )
                m1b = (
                    m1s[t % 2][:]
                    .rearrange("p (g s) -> p g s", s=S)[:, 0:G, :]
                    .unsqueeze(2)
                    .to_broadcast((P, G, C, S))
                )
                nc.vector.tensor_tensor(
                    out=h4[:, 0:G, 0:C, :],
                    in0=x4,
                    in1=m1b,
                    op=mybir.AluOpType.is_ge,
                ).then_inc(shd, 1)

            for t in range(NT):
                vector.wait_ge(sx, 16 * (t + 1))
                x3 = mview(xs[t % 4], CW, t)
                t8v = mview(t8, 48, t)
                t4v = mview(t4, 24, t)
                t2v = mview(t2, 12, t)
                m1v = mview(m1s[t % 2], 6, t)
                # (8,5-inplace,4,2,1) tree: 60 cyc/group
                nc.vector.tensor_tensor(
                    out=t8v, in0=x3[:, :, 0:48], in1=x3[:, :, 48:96], op=mx
                )
                nc.vector.tensor_tensor(
                    out=t8v[:, :, 0:30], in0=t8v[:, :, 0:30],
                    in1=x3[:, :, 96:126], op=mx,
                )
                if t >= 1:
                    # weave isge(t-1): independent, overlaps the DRAIN
                    isge(t - 1)
                nc.vector.tensor_tensor(
                    out=t4v, in0=t8v[:, :, 0:24], in1=t8v[:, :, 24:48], op=mx
                )
                nc.vector.tensor_tensor(
                    out=t2v, in0=t4v[:, :, 0:12], in1=t4v[:, :, 12:24], op=mx
                )
                nc.vector.tensor_tensor(
                    out=m1v, in0=t2v[:, :, 0:6], in1=t2v[:, :, 6:12], op=mx
                )
            isge(NT - 1)
            vector.wait_ge(sp, NT)
            nc.vector.tensor_copy(osb[:], cm_psum[:]).then_inc(aux, 1)

        @block.tensor
        def _(tensor):
            tensor.wait_ge(aux, 1)
            for t in range(NT):
                tensor.wait_ge(sy, 16 * (t + 1))
                tensor.wait_ge(shd, t + 1)
                for g in range(G_TILES[t]):
                    mm = nc.tensor.matmul(
                        out=cm_psum[:],
                        lhsT=ys[t % 3][:, g * YW : (g + 1) * YW],
                        rhs=hs[t % 3][:, g * HWC : (g + 1) * HWC],
                        start=(t == 0 and g == 0),
                        stop=(t == NT - 1 and g == G_TILES[t] - 1),
                    )
                mm.then_inc(sp, 1)

    return nc


def _get_nc():
    global _CACHED_NC
    if _CACHED_NC is None:
        _CACHED_NC = build_nc()
    return _CACHED_NC


def make_in_maps(input, target):
    inp = np.asarray(input, dtype=np.float32)
    tgt = np.asarray(target, dtype=np.float32)
    in_maps = []
    for b in range(B):
        xb = inp[b].reshape(C, N).T  # [N, C]
        xq = np.full((NPAD, C), NEG, dtype=np.float16)
        xq[:N] = xb
        # pad pixels keep x = NEG everywhere -> h row all-ones but y rows are 0
        # class-major: [P, NG, C, S]
        x_dev = np.ascontiguousarray(
            xq.reshape(NG, S, P, C).transpose(2, 0, 3, 1)
        ).reshape(P, NG * CW)

        yb = tgt[b].reshape(C, N).T  # [N, C]
        yq = np.zeros((NPAD, C), dtype=np.float32)
        yq[:N] = yb
        y4 = yq.reshape(NG, S, P, C).transpose(2, 0, 3, 1)  # [P,NG,C,S]
        y_dev = np.zeros((P, NG, YW), dtype=ml_dtypes.float8_e4m3)
        y_dev[..., :CW] = y4.reshape(P, NG, CW).astype(ml_dtypes.float8_e4m3)
        in_maps.append({"x": x_dev, "y": y_dev.reshape(P, NG * YW)})
    return in_maps


def postprocess(outs):
    final = np.zeros((C, C), dtype=np.float64)
    for o in outs:
        o = np.asarray(o, dtype=np.float64)  # [128, 132]
        ov = o[:CW, :CW].reshape(C, S, C, S)
        cm = np.einsum("isjs->ij", ov)
        rsv = o[:CW, CW:HWC].reshape(C, S, S)
        rs = np.einsum("iss->i", rsv).reshape(C, 1)
        final += cm / (rs + 1e-8)
    return (final / len(outs)).astype(np.float32)


def kernel(input, target):
    nc = _get_nc()
    in_maps = make_in_maps(input, target)
    res = run_bass_kernel_spmd(nc, in_maps, list(range(N_CORES)))
    return postprocess([r["out"] for r in res.results])


# revision 21
# speedup vs baseline: 1.0338x; 1.0338x over previous
"""Confusion-matrix kernel for Trainium2 (8 NeuronCores, data-parallel over batch).

Per batch b (one per core):
    pred[n]  = argmax_c input[b, c, n]            (n = pixel, N = H*W)
    cm[i, j] = sum_n target[b, i, n] * (pred[n] == j)
    rs[i]    = sum_n target[b, i, n]
Host: cm_b = cm / (rs + 1e-8); out = mean_b cm_b.

CLASS-MAJOR layout: per group (128 partitions x 6 pixel slots = 768 pixels),
x is stored as 21 class-blocks of 6 slot values -> 126 fp16 cols/group (no
pad column). The per-pixel max runs as an (8,5-inplace,4,2,1) block tree in
2x-mode tensor_tensor ops (60 cyc/group):
    T1: t8       = max(x[b0:b8],  x[b8:b16])
    T2: t8[0:5b] = max(t8[0:5b],  x[b16:b21])   (in-place accumulate)
    T3: t4       = max(t8[b0:b4], t8[b4:b8])
    T4: t2       = max(t4[b0:b2], t4[b2:b4])
    T5: m1       = max(t2[b0],    t2[b1])    -> per-slot max, 6 cols
    h[g, j, s] = is_ge(x[g, j, s], m1[g, s])  (63 cyc/group; bcast keeps 2x)
h is 132 cols/group: 126 one-hot + 6 ones cols (rs via matmul). The is_ge of
tile t-1 is woven between T2(t) and T3(t) so the dependent-chain DRAIN of
T2 overlaps an independent op.

One matmul per group: lhsT = y block [128, 128] fp8e4 (126 used),
rhs = h block [128, 132] fp16, accumulated into one [128, 132] f32 PSUM
tile; DVE does the final PSUM->SBUF copy. Host extracts
cm[i,j] = sum_s out[i*6+s, j*6+s], rs[i] = sum_s out[i*6+s, 126+s].

Buffering: x 4-deep, y 3-deep, h 3-deep; first y-load deferred until
isge(0) so the x stream gets full DMA bandwidth during the ramp. x/y DMA
completions are tracked by single cumulative semaphores (FIFO per queue);
5 semaphores total keeps the postamble sem-clear chain short.

Engines: SP x-loads + out-store | ACT y-loads | GPSIMD h ones-init |
DVE tree + is_ge + final copy | PE matmuls.
"""

from contextlib import ExitStack

import ml_dtypes
import numpy as np

import concourse.bass as bass
import concourse.mybir as mybir
from concourse.bass_utils import run_bass_kernel_spmd

B, C, H, W = 8, 21, 512, 512
N = H * W              # 262144 pixels per batch
P = 128                # SBUF partitions
S = 6                  # pixel slots per PE row
CW = C * S             # x cols per group (126), class-major
YW = 128               # y group width: 126 + 2 pad cols
HWC = (C + 1) * S      # h cols per group (132): 126 one-hot + 6 ones
NG = 342               # total groups per core (342*768 = 262656 >= N)
NPAD = NG * P * S      # padded pixel count
G_TILES = [10, 22, 44, 57, 57, 57, 57, 26, 12]
NT = len(G_TILES)
G_OFF = [sum(G_TILES[:i]) for i in range(NT)]
GMAX = max(G_TILES)
NEG = -65504.0
N_CORES = 8

_CACHED_NC = None


def build_nc():
    nc = bass.Bass()
    x = nc.declare_dram_parameter("x", [P, NG * CW], mybir.dt.float16, isOutput=False)
    y = nc.declare_dram_parameter("y", [P, NG * YW], mybir.dt.float8e4, isOutput=False)
    out = nc.declare_dram_parameter("out", [P, HWC], mybir.dt.float32, isOutput=True)

    mx = mybir.AluOpType.max

    with ExitStack() as ctx:
        xs = [
            ctx.enter_context(
                nc.sbuf_tensor(f"xsb{i}", [P, GMAX * CW], mybir.dt.float16)
            )
            for i in range(4)
        ]
        ys = [
            ctx.enter_context(
                nc.sbuf_tensor(f"ysb{i}", [P, GMAX * YW], mybir.dt.float8e4)
            )
            for i in range(3)
        ]
        hs = [
            ctx.enter_context(
                nc.sbuf_tensor(f"hsb{i}", [P, GMAX * HWC], mybir.dt.float16)
            )
            for i in range(3)
        ]
        t8 = ctx.enter_context(nc.sbuf_tensor("t8b", [P, GMAX * 48], mybir.dt.float16))
        t4 = ctx.enter_context(nc.sbuf_tensor("t4b", [P, GMAX * 24], mybir.dt.float16))
        t2 = ctx.enter_context(nc.sbuf_tensor("t2b", [P, GMAX * 12], mybir.dt.float16))
        m1s = [
            ctx.enter_context(nc.sbuf_tensor(f"m1b{i}", [P, GMAX * 6], mybir.dt.float16))
            for i in range(2)
        ]
        osb = ctx.enter_context(nc.sbuf_tensor("osb", [P, HWC], mybir.dt.float32))
        cm_psum = ctx.enter_context(nc.psum_tensor("cmps", [P, HWC], mybir.dt.float32))

        block = ctx.enter_context(nc.Block())
        sx = ctx.enter_context(nc.semaphore("sx"))      # x loads done (FIFO), 16/tile
        sy = ctx.enter_context(nc.semaphore("sy"))      # y loads done (FIFO), 16/tile
        shd = ctx.enter_context(nc.semaphore("shd"))    # DVE isge(t) done, = t+1
        sp = ctx.enter_context(nc.semaphore("sp"))      # PE tile matmuls done, = t+1
        aux = ctx.enter_context(nc.semaphore("aux"))    # 1=ones-init, 2=+copy, 18=+out-DMA

        def mview(buf, w, t):
            return (
                buf[:]
                .rearrange("p (g c) -> p g c", c=w)[:, 0 : G_TILES[t], :]
            )

        @block.sync
        def _(sync):
            for t in range(NT):
                if t >= 4:
                    sync.wait_ge(shd, t - 3)  # isge(t-4) freed x slot
                cols = G_TILES[t] * CW
                sync.dma_start(
                    out=xs[t % 4][:, 0:cols],
                    in_=x[:, G_OFF[t] * CW : G_OFF[t] * CW + cols],
                ).then_inc(sx, 16)
            sync.wait_ge(aux, 2)
            sync.dma_start(out=out[:], in_=osb[:]).then_inc(aux, 16)
            sync.wait_ge(aux, 18)

        @block.scalar
        def _(scalar):
            # defer the first y-loads until isge(0) is done: x feeds the DVE
            # ramp and gets the full DMA bandwidth until then.
            scalar.wait_ge(shd, 1)
            for t in range(NT):
                if t >= 3:
                    scalar.wait_ge(sp, t - 2)  # matmul(t-3) freed y slot
                cols = G_TILES[t] * YW
                scalar.dma_start(
                    out=ys[t % 3][:, 0:cols],
                    in_=y[:, G_OFF[t] * YW : G_OFF[t] * YW + cols],
                ).then_inc(sy, 16)


        @block.gpsimd
        def _(gpsimd):
            h3v = [h[:].rearrange("p (g w) -> p g w", w=HWC) for h in hs]
            nc.gpsimd.memset(h3v[0][:, :, CW:HWC], 1.0)
            nc.gpsimd.memset(h3v[1][:, :, CW:HWC], 1.0)
            nc.gpsimd.memset(h3v[2][:, :, CW:HWC], 1.0).then_inc(aux, 1)

        @block.vector
        def _(vector):
            def isge(t):
                # h(t) one-hot: compare x against per-slot max, broadcast
                # over the 21 class blocks (inner step 1 keeps 2x mode)
                if t >= 3:
                    vector.wait_ge(sp, t - 2)  # matmul(t-3) freed h slot
                G = G_TILES[t]
                x4 = (
                    xs[t % 4][:]
                    .rearrange("p (g j s) -> p g j s", j=C, s=S)[:, 0:G, :, :]
                )
                h4 = (
                    hs[t % 3][:]
                    .rearrange("p (g j s) -> p g j s", j=C + 1, s=S)
                )
                m1b = (
                    m1s[t % 2][:]
                    .rearrange("p (g s) -> p g s", s=S)[:, 0:G, :]
                    .unsqueeze(2)
                    .to_broadcast((P, G, C, S))
                )
                nc.vector.tensor_tensor(
                    out=h4[:, 0:G, 0:C, :],
                    in0=x4,
                    in1=m1b,
                    op=mybir.AluOpType.is_ge,
                ).then_inc(shd, 1)

            for t in range(NT):
                vector.wait_ge(sx, 16 * (t + 1))
                x3 = mview(xs[t % 4], CW, t)
                t8v = mview(t8, 48, t)
                t4v = mview(t4, 24, t)
                t2v = mview(t2, 12, t)
                m1v = mview(m1s[t % 2], 6, t)
                # (8,5-inplace,4,2,1) tree: 60 cyc/group
                nc.vector.tensor_tensor(
                    out=t8v, in0=x3[:, :, 0:48], in1=x3[:, :, 48:96], op=mx
                )
                nc.vector.tensor_tensor(
                    out=t8v[:, :, 0:30], in0=t8v[:, :, 0:30],
                    in1=x3[:, :, 96:126], op=mx,
                )
                if t >= 1:
                    # weave isge(t-1): independent, overlaps the DRAIN
                    isge(t - 1)
                nc.vector.tensor_tensor(
                    out=t4v, in0=t8v[:, :, 0:24], in1=t8v[:, :, 24:48], op=mx
                )
                nc.vector.tensor_tensor(
                    out=t2v, in0=t4v[:, :, 0:12], in1=t4v[:, :, 12:24], op=mx
                )
                nc.vector.tensor_tensor(
                    out=m1v, in0=t2v[:, :, 0:6], in1=t2v[:, :, 6:12], op=mx
                )
            isge(NT - 1)
            vector.wait_ge(sp, NT)
            nc.vector.tensor_copy(osb[:], cm_psum[:]).then_inc(aux, 1)

        @block.tensor
        def _(tensor):
            tensor.wait_ge(aux, 1)
            for t in range(NT):
                tensor.wait_ge(sy, 16 * (t + 1))
                tensor.wait_ge(shd, t + 1)
                for g in range(G_TILES[t]):
                    mm = nc.tensor.matmul(
                        out=cm_psum[:],
                        lhsT=ys[t % 3][:, g * YW : (g + 1) * YW],
                        rhs=hs[t % 3][:, g * HWC : (g + 1) * HWC],
                        start=(t == 0 and g == 0),
                        stop=(t == NT - 1 and g == G_TILES[t] - 1),
                    )
                mm.then_inc(sp, 1)

    return nc


def _get_nc():
    global _CACHED_NC
    if _CACHED_NC is None:
        _CACHED_NC = build_nc()
    return _CACHED_NC


def make_in_maps(input, target):
    inp = np.asarray(input, dtype=np.float32)
    tgt = np.asarray(target, dtype=np.float32)
    in_maps = []
    for b in range(B):
        xb = inp[b].reshape(C, N).T  # [N, C]
        xq = np.full((NPAD, C), NEG, dtype=np.float16)
        xq[:N] = xb
        # pad pixels keep x = NEG everywhere -> h row all-ones but y rows are 0
        # class-major: [P, NG, C, S]
        x_dev = np.ascontiguousarray(
            xq.reshape(NG, S, P, C).transpose(2, 0, 3, 1)
        ).reshape(P, NG * CW)

        yb = tgt[b].reshape(C, N).T  # [N, C]
        yq = np.zeros((NPAD, C), dtype=np.float32)
        yq[:N] = yb
        y4 = yq.reshape(NG, S, P, C).transpose(2, 0, 3, 1)  # [P,NG,C,S]
        y_dev = np.zeros((P, NG, YW), dtype=ml_dtypes.float8_e4m3)
        y_dev[..., :CW] = y4.reshape(P, NG, CW).astype(ml_dtypes.float8_e4m3)
        in_maps.append({"x": x_dev, "y": y_dev.reshape(P, NG * YW)})
    return in_maps


def postprocess(outs):
    final = np.zeros((C, C), dtype=np.float64)
    for o in outs:
        o = np.asarray(o, dtype=np.float64)  # [128, 132]
        ov = o[:CW, :CW].reshape(C, S, C, S)
        cm = np.einsum("isjs->ij", ov)
        rsv = o[:CW, CW:HWC].reshape(C, S, S)
        rs = np.einsum("iss->i", rsv).reshape(C, 1)
        final += cm / (rs + 1e-8)
    return (final / len(outs)).astype(np.float32)


def kernel(input, target):
    nc = _get_nc()
    in_maps = make_in_maps(input, target)
    res = run_bass_kernel_spmd(nc, in_maps, list(range(N_CORES)))
    return postprocess([r["out"] for r in res.results])
